# revision 32
# baseline (speedup 1.0000x reference)
"""TISA-biased multi-head attention kernel for 8 TRN2 NeuronCores.

Strategy (self-contained, full inputs in / full output out):
  - Shard: core i handles batch b=i//4, query rows (i%4)*512..+512 (all heads).
  - Per-head TISA RBF bias sum_f a*exp(-|b|*(d-c)^2) is replaced by
    exp(poly3_h(d)) with poly fit per head on the host (max bias err ~5e-3),
    where d = sqrt(min(d^2, DCUT^2)) and d^2 comes from a tiny fp32 matmul
    over location coordinates.
  - Scores are computed transposed [k, q] so softmax sums ride the PV matmul
    (ones column appended to V) and no transposes are needed anywhere.
  - No max-subtraction in softmax (scores bounded ~12); global -SHIFT keeps
    the fp16 numerator in range and cancels in the normalization.
  - fp16 everywhere on 16-bit tensors (8x the mantissa of bf16, same speed).
  - Key masking (valid_lens) by zeroing masked keys' rows (and their ones-
    column entries) in the augmented V, so masked keys contribute to neither
    PV nor the softmax sums; program is uniform across cores, masking is data.
"""

import numpy as np
from contextlib import ExitStack

import concourse.bacc as bacc
import concourse.tile as tile
from concourse import mybir
from concourse.bass_utils import run_bass_kernel_spmd

B, Q, K, DM, H, F = 2, 2048, 2048, 512, 8, 5
DH = DM // H          # 64
QC = Q // 4           # 512 queries per core
KT = K // 128         # 16 key tiles
NCB = 5               # contraction chunks of 128 for 640-row augmented inputs
DEG = 3
DCUT = 3.7
SHIFT = 6.0
MASKVAL = -30000.0

f16 = mybir.dt.float16
f32 = mybir.dt.float32
AF = mybir.ActivationFunctionType
OP = mybir.AluOpType


def _fit_lnbias(a, b, c):
    """Per-head degree-DEG polynomial fit of ln(sum_f a*exp(-b(d-c)^2)) on
    [0, DCUT]. Returns [H, DEG+1] coefficients, increasing order."""
    a = a.reshape(H, F).astype(np.float64)
    b = np.abs(b.reshape(H, F)).astype(np.float64)
    c = c.reshape(H, F).astype(np.float64)
    n = 4000
    d = (np.cos(np.pi * (np.arange(n) + 0.5) / n) + 1) / 2 * DCUT
    out = np.zeros((H, DEG + 1))
    for h in range(H):
        g = (a[h][:, None] * np.exp(-b[h][:, None] * (d[None, :] - c[h][:, None]) ** 2)).sum(0)
        w = np.sqrt(np.maximum(g, 3e-4))
        V = np.vander(d, DEG + 1, increasing=True) * w[:, None]
        coef, *_ = np.linalg.lstsq(V, np.log(g) * w, rcond=None)
        out[h] = coef
    return out


def _chunk(x):
    """[640, N] -> [128, 5, N] (row cb*128+p -> [p, cb])."""
    n = x.shape[1]
    return np.ascontiguousarray(x.reshape(NCB, 128, n).transpose(1, 0, 2))


def _build_nc(dbg=False):
    nc = bacc.Bacc("TRN2", target_bir_lowering=False, debug=False,
                   enable_asserts=False, num_devices=8)
    t_in = {}
    for name, shape, dt in [
        ("qsT", (128, NCB, QC), f16), ("ksT", (128, 4, K), f16),
        ("vsT", (128, 4, K), f16),
        ("wq", (128, NCB, DM), f16), ("wk", (128, 4, DM), f16),
        ("wv", (128, 4, DM), f16), ("wo", (64, H, DM), f16),
        ("qlocT", (4, QC), f32), ("klocT", (4, K), f32),
        ("nk", (128, KT), f32), ("kbias", (128, KT), f32), ("vmask", (128, KT), f32),
        ("coeffs", (128, 4 * H), f32), ("ident", (128, 128), f16),
        ("onesf", (1, 64), f32),
    ]:
        t_in[name] = nc.dram_tensor(name, shape, dt, kind="ExternalInput")
    t_out = nc.dram_tensor("out", (4, 128, DM), f32, kind="ExternalOutput")
    if dbg:
        t_dsums = nc.dram_tensor("dbg_sums", (H, QC), f32, kind="ExternalOutput")
        t_dqt = nc.dram_tensor("dbg_qt", (64, H, QC), f16, kind="ExternalOutput")
        t_ddc = nc.dram_tensor("dbg_dc", (128, KT, QC), f16, kind="ExternalOutput")
        t_dctx = nc.dram_tensor("dbg_ctx", (64, H, QC), f16, kind="ExternalOutput")

    with ExitStack() as ctx:
        tc = ctx.enter_context(tile.TileContext(nc))
        cp = ctx.enter_context(tc.tile_pool(name="consts", bufs=1))
        pp = ctx.enter_context(tc.tile_pool(name="persist", bufs=1))

        qT = pp.tile([128, H // 2, QC], f16)
        kT = pp.tile([128, H // 2, K], f16)
        v_aug = pp.tile([128, KT, H, 65], f16)
        dc16 = pp.tile([128, KT, QC], f16)
        ctx_sb = pp.tile([64, H, QC], f16)
        wo = pp.tile([64, H, DM], f16)

        coeffs = cp.tile([128, 4 * H], f32)
        vmask = cp.tile([128, KT], f32)
        nk = cp.tile([128, KT], f32)
        kbias = cp.tile([128, KT], f32)
        ident = cp.tile([128, 128], f16)
        qlocT = cp.tile([4, QC], f32)
        klocT = cp.tile([4, K], f32)
        onesf = cp.tile([1, 64], f32)
        # ---- Phase 1: projections -------------------------------------
        with tc.tile_pool(name="p1in", bufs=1) as p1, \
             tc.tile_pool(name="p1ps", bufs=4, space="PSUM") as mmp:
            qsT = p1.tile([128, NCB, QC], f16)
            ksT = p1.tile([128, 4, K], f16)
            vsT = p1.tile([128, 4, K], f16)
            wq = p1.tile([128, NCB, DM], f16)
            wk = p1.tile([128, 4, DM], f16)
            wv = p1.tile([128, 4, DM], f16)
            for eng, tl, name in [(nc.sync, wq, "wq"), (nc.scalar, qsT, "qsT"),
                                  (nc.sync, wk, "wk"), (nc.sync, ksT, "ksT"),
                                  (nc.scalar, wv, "wv"), (nc.scalar, vsT, "vsT")]:
                eng.dma_start(out=tl, in_=t_in[name][:, :, :])
            for tl, name in [(vmask, "vmask"), (ident, "ident"), (qlocT, "qlocT"),
                             (klocT, "klocT"), (nk, "nk"), (coeffs, "coeffs"),
                             (kbias, "kbias"), (onesf, "onesf")]:
                nc.scalar.dma_start(out=tl, in_=t_in[name][:, :] if len(tl.shape) == 2 else t_in[name][:, :, :])

            for mb in range(4):
                ps = mmp.tile([128, QC], f32, tag="ps")
                for cb in range(NCB):
                    nc.tensor.matmul(ps, lhsT=wq[:, cb, mb * 128:(mb + 1) * 128],
                                     rhs=qsT[:, cb, :],
                                     start=(cb == 0), stop=(cb == NCB - 1))
                nc.vector.tensor_copy(qT[:, mb, :], ps)
                for nb in range(K // 512):
                    ps2 = mmp.tile([128, 512], f32, tag="ps")
                    for cb in range(4):
                        nc.tensor.matmul(ps2, lhsT=wk[:, cb, mb * 128:(mb + 1) * 128],
                                         rhs=ksT[:, cb, nb * 512:(nb + 1) * 512],
                                         start=(cb == 0), stop=(cb == 3))
                    nc.vector.tensor_copy(kT[:, mb, nb * 512:(nb + 1) * 512], ps2)
            for kt in range(KT):
                ps3 = mmp.tile([128, DM], f32, tag="psv")
                for cb in range(4):
                    nc.tensor.matmul(ps3, lhsT=vsT[:, cb, kt * 128:(kt + 1) * 128],
                                     rhs=wv[:, cb, :],
                                     start=(cb == 0), stop=(cb == 3))
                nc.scalar.copy(
                    v_aug[:, kt, :, 0:64],
                    ps3.rearrange("p (h d) -> p h d", h=H))
            nc.vector.memset(v_aug[:, :, :, 64:65], 1.0)
            for kt in range(KT):
                vv = v_aug[:, kt, :, :].rearrange("p h d -> p (h d)")
                nc.vector.tensor_scalar_mul(vv, vv, vmask[:, kt:kt + 1])

        # ---- Phase 2: distances ---------------------------------------
        with tc.tile_pool(name="p2ps", bufs=2, space="PSUM") as gps, \
             tc.tile_pool(name="p2sb", bufs=2) as d2p:
            for kt in range(KT):
                gp = gps.tile([128, QC], f32)
                nc.tensor.matmul(gp, lhsT=klocT[:, kt * 128:(kt + 1) * 128],
                                 rhs=qlocT[:, :], start=True, stop=True)
                d2 = d2p.tile([128, QC], f32)
                nc.vector.tensor_scalar(d2, gp, nk[:, kt:kt + 1],
                                        DCUT * DCUT, OP.add, OP.min)
                nc.scalar.activation(dc16[:, kt, :], d2, AF.Sqrt)

        nc.sync.dma_start(out=wo, in_=t_in["wo"][:, :, :])

        # ---- Phase 3: attention per head ------------------------------
        with tc.tile_pool(name="sps", bufs=5, space="PSUM") as sps, \
             tc.tile_pool(name="ctxps", bufs=2, space="PSUM") as cps, \
             tc.tile_pool(name="rbps", bufs=1, space="PSUM") as rps, \
             tc.tile_pool(name="p3x", bufs=3) as xp, \
             tc.tile_pool(name="p3e", bufs=3) as ep, \
             tc.tile_pool(name="p3p", bufs=4) as ptp, \
             tc.tile_pool(name="p3r", bufs=2) as rp:
            for h in range(H):
                ctxp = cps.tile([65, QC], f32, tag="ctx")
                for g in range(4):
                    dcg = dc16[:, 4 * g:4 * g + 4, :]
                    t1 = xp.tile([128, 4, QC], f16, tag="t1")
                    nc.vector.tensor_scalar_add(t1, dcg, coeffs[:, 4 * h:4 * h + 1])
                    x1 = xp.tile([128, 4, QC], f16, tag="x1")
                    nc.vector.tensor_tensor(out=x1, in0=t1, in1=dcg, op=OP.mult)
                    t2 = xp.tile([128, 4, QC], f16, tag="t2")
                    nc.vector.tensor_scalar_add(t2, x1, coeffs[:, 4 * h + 1:4 * h + 2])
                    x2 = xp.tile([128, 4, QC], f16, tag="x2")
                    nc.vector.tensor_tensor(out=x2, in0=t2, in1=dcg, op=OP.mult)
                    ee = ep.tile([128, 4, QC], f16, tag="ee")
                    nc.scalar.activation(ee, x2, AF.Exp,
                                         bias=coeffs[:, 4 * h + 3:4 * h + 4],
                                         scale=coeffs[:, 4 * h + 2:4 * h + 3])
                    for j in range(4):
                        kt = 4 * g + j
                        po = (h % 2) * 64
                        sp = sps.tile([128, QC], f32, tag="sp")
                        nc.tensor.matmul(sp, lhsT=kT[po:po + 64, h // 2,
                                                     kt * 128:(kt + 1) * 128],
                                         rhs=qT[po:po + 64, h // 2, :],
                                         start=True, stop=False)
                        nc.tensor.matmul(sp, lhsT=ident, rhs=ee[:, j, :],
                                         start=False, stop=True)
                        pt = ptp.tile([128, QC], f16, tag="pt")
                        nc.scalar.activation(pt, sp, AF.Exp,
                                             bias=kbias[:, kt:kt + 1])
                        nc.tensor.matmul(ctxp, lhsT=v_aug[:, kt, h, :], rhs=pt,
                                         start=(kt == 0), stop=(kt == KT - 1))
                scop = rp.tile([1, QC], f32, tag="scop")
                nc.vector.tensor_copy(scop, ctxp[64:65, :])
                rinv = rp.tile([1, QC], f32, tag="rinv")
                nc.vector.reciprocal_approx_fast(rinv, scop)
                if dbg:
                    dsum = rp.tile([1, QC], f32, tag="dsum")
                    nc.vector.tensor_copy(dsum, ctxp[64:65, :])
                    nc.sync.dma_start(out=t_dsums[h:h + 1, :], in_=dsum)
                rbp = rps.tile([64, QC], f32, tag="rb")
                nc.tensor.matmul(rbp, lhsT=onesf, rhs=rinv, start=True, stop=True)
                rb = rp.tile([64, QC], f32, tag="rbsb")
                nc.vector.tensor_copy(rb, rbp)
                nc.vector.tensor_tensor(out=ctx_sb[:, h, :], in0=ctxp[0:64, :],
                                        in1=rb, op=OP.mult)

        if dbg:
            nc.sync.dma_start(out=t_dqt[:, :, :], in_=qT)
            nc.sync.dma_start(out=t_ddc[:, :, :], in_=dc16)
            nc.sync.dma_start(out=t_dctx[:, :, :], in_=ctx_sb)

        # ---- Phase 4: output projection -------------------------------
        with tc.tile_pool(name="ops", bufs=2, space="PSUM") as ops, \
             tc.tile_pool(name="osb", bufs=2) as osb:
            for qc in range(4):
                op = ops.tile([128, DM], f32, tag="op")
                for h in range(H):
                    nc.tensor.matmul(op, lhsT=ctx_sb[:, h, qc * 128:(qc + 1) * 128],
                                     rhs=wo[:, h, :], start=(h == 0), stop=(h == H - 1))
                ot = osb.tile([128, DM], f32, tag="ot")
                nc.vector.tensor_copy(ot, op)
                nc.sync.dma_start(out=t_out[qc, :, :], in_=ot)

    nc.compile()
    return nc


_NC_CACHE = None


def _get_nc():
    global _NC_CACHE
    if _NC_CACHE is None:
        _NC_CACHE = _build_nc()
    return _NC_CACHE


def _prepare_in_maps(qs, ks, vs, qs_locs, ks_locs, Wq, bq, Wk, bk, Wv, bv,
                     Wo, bo, a, b, c, valid_lens):
    qs, ks, vs = np.asarray(qs, np.float32), np.asarray(ks, np.float32), np.asarray(vs, np.float32)
    qs_locs, ks_locs = np.asarray(qs_locs, np.float32), np.asarray(ks_locs, np.float32)
    vl = np.asarray(valid_lens).astype(np.int64)

    coefs = _fit_lnbias(np.asarray(a), np.asarray(b), np.asarray(c))
    crow = np.zeros(4 * H, np.float32)
    for h in range(H):
        s = coefs[h, DEG]
        crow[4 * h + 0] = coefs[h, 2] / s
        crow[4 * h + 1] = coefs[h, 1] / s
        crow[4 * h + 2] = s
        crow[4 * h + 3] = coefs[h, 0]
    coeffs_t = np.tile(crow, (128, 1)).astype(np.float32)

    def aug(x, last_row):
        A = np.zeros((NCB * 128, x.shape[1]), np.float32)
        A[:x.shape[0]] = x
        A[x.shape[0]] = last_row
        return _chunk(A.astype(np.float16))

    def chunk4(x):
        return np.ascontiguousarray(
            np.asarray(x, np.float32).astype(np.float16).reshape(4, 128, x.shape[1]).transpose(1, 0, 2))

    # bk shifts every score of a given query equally -> softmax-invariant, dropped.
    # bv is folded into bo on the host: (ctx + bv) @ Wo = ctx @ Wo + bv @ Wo.
    wq_c = aug(np.asarray(Wq, np.float32) / 8.0, np.asarray(bq, np.float32) / 8.0)
    wk_c = chunk4(np.asarray(Wk, np.float32))
    wv_c = chunk4(np.asarray(Wv, np.float32))
    wo_c = np.ascontiguousarray(
        np.asarray(Wo, np.float32).astype(np.float16).reshape(H, 64, DM).transpose(1, 0, 2))
    ident = np.eye(128, dtype=np.float16)
    onesf = np.ones((1, 64), np.float32)

    in_maps = []
    for i in range(8):
        bi, qo = i // 4, (i % 4) * QC
        qsl = qs[bi, qo:qo + QC]
        ksl, vsl = ks[bi], vs[bi]
        ql, kl = qs_locs[bi, qo:qo + QC], ks_locs[bi]
        qlocT = np.stack([ql[:, 0], ql[:, 1], (ql ** 2).sum(1),
                          np.zeros(QC, np.float32)]).astype(np.float32)
        klocT = np.stack([-2.0 * kl[:, 0], -2.0 * kl[:, 1],
                          np.ones(K, np.float32), np.zeros(K, np.float32)]).astype(np.float32)
        # +4e-4 keeps d^2 strictly positive under fp32 matmul cancellation
        # (sqrt(negative) = NaN); the d shift of <=0.02 near d=0 is harmless.
        nk_t = np.ascontiguousarray(
            ((kl ** 2).sum(1) + 4e-4).reshape(KT, 128).T.astype(np.float32))
        kb_t = np.full((128, KT), -SHIFT, np.float32)
        vm = (np.arange(K) < vl[bi]).astype(np.float32)
        vm_t = np.ascontiguousarray(vm.reshape(KT, 128).T)
        in_maps.append({
            "qsT": aug(qsl.T, 1.0), "ksT": chunk4(ksl.T), "vsT": chunk4(vsl.T),
            "wq": wq_c, "wk": wk_c, "wv": wv_c, "wo": wo_c,
            "qlocT": qlocT, "klocT": klocT, "nk": nk_t, "kbias": kb_t, "vmask": vm_t,
            "coeffs": coeffs_t, "ident": ident, "onesf": onesf,
        })
    return in_maps


def kernel(qs, ks, vs, qs_locs, ks_locs, Wq, bq, Wk, bk, Wv, bv, Wo, bo,
           a, b, c, valid_lens, **_unused):
    import os
    in_maps = _prepare_in_maps(qs, ks, vs, qs_locs, ks_locs, Wq, bq, Wk, bk,
                               Wv, bv, Wo, bo, a, b, c, valid_lens)
    nc = _get_nc()
    trace = bool(os.environ.get("KERNEL_TRACE"))
    res = run_bass_kernel_spmd(nc, in_maps, core_ids=list(range(8)), trace=trace)
    if trace and res.exec_time_ns is not None:
        print(f"HW exec time: {res.exec_time_ns} ns")

    bo32 = (np.asarray(bo, np.float32) +
            np.asarray(bv, np.float32) @ np.asarray(Wo, np.float32))
    out = np.zeros((B, Q, DM), np.float32)
    for i in range(8):
        bi, qo = i // 4, (i % 4) * QC
        out[bi, qo:qo + QC] = res.results[i]["out"].reshape(QC, DM) + bo32[None, :]
    return out


# revision 33
# speedup vs baseline: 1.0401x; 1.0401x over previous
"""TISA-biased multi-head attention kernel for 8 TRN2 NeuronCores.

Strategy (self-contained, full inputs in / full output out):
  - Shard: core i handles batch b=i//4, query rows (i%4)*512..+512 (all heads).
  - Per-head TISA RBF bias sum_f a*exp(-|b|*(d-c)^2) is replaced by
    exp(poly3_h(d)) with poly fit per head on the host (max bias err ~5e-3),
    where d = sqrt(min(d^2, DCUT^2)) and d^2 comes from a tiny fp32 matmul
    over location coordinates.
  - Scores are computed transposed [k, q] so softmax sums ride the PV matmul
    (ones column appended to V) and no transposes are needed anywhere.
  - No max-subtraction in softmax (scores bounded ~12); global -SHIFT keeps
    the fp16 numerator in range and cancels in the normalization.
  - fp16 everywhere on 16-bit tensors (8x the mantissa of bf16, same speed).
  - Key masking (valid_lens) by zeroing masked keys' rows (and their ones-
    column entries) in the augmented V, so masked keys contribute to neither
    PV nor the softmax sums; program is uniform across cores, masking is data.
"""

import numpy as np
from contextlib import ExitStack

import concourse.bacc as bacc
import concourse.tile as tile
from concourse import mybir
from concourse.bass_utils import run_bass_kernel_spmd

B, Q, K, DM, H, F = 2, 2048, 2048, 512, 8, 5
DH = DM // H          # 64
QC = Q // 4           # 512 queries per core
KT = K // 128         # 16 key tiles
NCB = 5               # contraction chunks of 128 for 640-row augmented inputs
DEG = 3
DCUT = 3.7
SHIFT = 6.0
MASKVAL = -30000.0

f16 = mybir.dt.float16
f32 = mybir.dt.float32
AF = mybir.ActivationFunctionType
OP = mybir.AluOpType


def _fit_lnbias(a, b, c):
    """Per-head degree-DEG polynomial fit of ln(sum_f a*exp(-b(d-c)^2)) on
    [0, DCUT]. Returns [H, DEG+1] coefficients, increasing order."""
    a = a.reshape(H, F).astype(np.float64)
    b = np.abs(b.reshape(H, F)).astype(np.float64)
    c = c.reshape(H, F).astype(np.float64)
    n = 4000
    d = (np.cos(np.pi * (np.arange(n) + 0.5) / n) + 1) / 2 * DCUT
    out = np.zeros((H, DEG + 1))
    for h in range(H):
        g = (a[h][:, None] * np.exp(-b[h][:, None] * (d[None, :] - c[h][:, None]) ** 2)).sum(0)
        w = np.sqrt(np.maximum(g, 3e-4))
        V = np.vander(d, DEG + 1, increasing=True) * w[:, None]
        coef, *_ = np.linalg.lstsq(V, np.log(g) * w, rcond=None)
        out[h] = coef
    return out


def _chunk(x):
    """[640, N] -> [128, 5, N] (row cb*128+p -> [p, cb])."""
    n = x.shape[1]
    return np.ascontiguousarray(x.reshape(NCB, 128, n).transpose(1, 0, 2))


def _build_nc(dbg=False):
    nc = bacc.Bacc("TRN2", target_bir_lowering=False, debug=False,
                   enable_asserts=False, num_devices=8)
    t_in = {}
    for name, shape, dt in [
        ("qsT", (128, NCB, QC), f16), ("ksT", (128, 4, K), f16),
        ("vsT", (128, 4, K), f16),
        ("wq", (128, NCB, DM), f16), ("wk", (128, 4, DM), f16),
        ("wv", (128, 4, DM), f16), ("wo", (64, H, DM), f16),
        ("qlocT", (4, QC), f32), ("klocT", (4, K), f32),
        ("nk", (128, KT), f32), ("kbias", (128, KT), f32), ("vmask", (128, KT), f32),
        ("coeffs", (128, 4 * H), f32), ("ident", (128, 128), f16),
        ("onesf", (1, 64), f32),
    ]:
        t_in[name] = nc.dram_tensor(name, shape, dt, kind="ExternalInput")
    t_out = nc.dram_tensor("out", (4, 128, DM), f32, kind="ExternalOutput")
    if dbg:
        t_dsums = nc.dram_tensor("dbg_sums", (H, QC), f32, kind="ExternalOutput")
        t_dqt = nc.dram_tensor("dbg_qt", (64, H, QC), f16, kind="ExternalOutput")
        t_ddc = nc.dram_tensor("dbg_dc", (128, KT, QC), f16, kind="ExternalOutput")
        t_dctx = nc.dram_tensor("dbg_ctx", (64, H, QC), f16, kind="ExternalOutput")

    with ExitStack() as ctx:
        tc = ctx.enter_context(tile.TileContext(nc))
        cp = ctx.enter_context(tc.tile_pool(name="consts", bufs=1))
        pp = ctx.enter_context(tc.tile_pool(name="persist", bufs=1))

        qT = pp.tile([128, H // 2, QC], f16)
        kT = pp.tile([128, H // 2, K], f16)
        v_aug = pp.tile([128, KT, H, 65], f16)
        dc16 = pp.tile([128, KT, QC], f16)
        ctx_sb = pp.tile([64, H, QC], f16)
        wo = pp.tile([64, H, DM], f16)

        coeffs = cp.tile([128, 4 * H], f32)
        vmask = cp.tile([128, KT], f32)
        nk = cp.tile([128, KT], f32)
        kbias = cp.tile([128, KT], f32)
        ident = cp.tile([128, 128], f16)
        qlocT = cp.tile([4, QC], f32)
        klocT = cp.tile([4, K], f32)
        onesf = cp.tile([1, 64], f32)
        # ---- Phase 1: projections -------------------------------------
        with tc.tile_pool(name="p1in", bufs=1) as p1, \
             tc.tile_pool(name="p1ps", bufs=4, space="PSUM") as mmp:
            qsT = p1.tile([128, NCB, QC], f16)
            ksT = p1.tile([128, 4, K], f16)
            vsT = p1.tile([128, 4, K], f16)
            wq = p1.tile([128, NCB, DM], f16)
            wk = p1.tile([128, 4, DM], f16)
            wv = p1.tile([128, 4, DM], f16)
            for eng, tl, name in [(nc.sync, wq, "wq"), (nc.sync, qsT, "qsT"),
                                  (nc.sync, wk, "wk"), (nc.sync, ksT, "ksT"),
                                  (nc.scalar, wv, "wv"), (nc.scalar, vsT, "vsT")]:
                eng.dma_start(out=tl, in_=t_in[name][:, :, :])
            for tl, name in [(vmask, "vmask"), (ident, "ident"), (qlocT, "qlocT"),
                             (klocT, "klocT"), (nk, "nk"), (coeffs, "coeffs"),
                             (kbias, "kbias"), (onesf, "onesf")]:
                nc.scalar.dma_start(out=tl, in_=t_in[name][:, :] if len(tl.shape) == 2 else t_in[name][:, :, :])

            for mb in range(4):
                ps = mmp.tile([128, QC], f32, tag="ps")
                for cb in range(NCB):
                    nc.tensor.matmul(ps, lhsT=wq[:, cb, mb * 128:(mb + 1) * 128],
                                     rhs=qsT[:, cb, :],
                                     start=(cb == 0), stop=(cb == NCB - 1))
                nc.vector.tensor_copy(qT[:, mb, :], ps)
                for nb in range(K // 512):
                    ps2 = mmp.tile([128, 512], f32, tag="ps")
                    for cb in range(4):
                        nc.tensor.matmul(ps2, lhsT=wk[:, cb, mb * 128:(mb + 1) * 128],
                                         rhs=ksT[:, cb, nb * 512:(nb + 1) * 512],
                                         start=(cb == 0), stop=(cb == 3))
                    nc.vector.tensor_copy(kT[:, mb, nb * 512:(nb + 1) * 512], ps2)
            for kt in range(KT):
                ps3 = mmp.tile([128, DM], f32, tag="psv")
                for cb in range(4):
                    nc.tensor.matmul(ps3, lhsT=vsT[:, cb, kt * 128:(kt + 1) * 128],
                                     rhs=wv[:, cb, :],
                                     start=(cb == 0), stop=(cb == 3))
                nc.scalar.copy(
                    v_aug[:, kt, :, 0:64],
                    ps3.rearrange("p (h d) -> p h d", h=H))
            nc.vector.memset(v_aug[:, :, :, 64:65], 1.0)
            for kt in range(KT):
                vv = v_aug[:, kt, :, :].rearrange("p h d -> p (h d)")
                nc.vector.tensor_scalar_mul(vv, vv, vmask[:, kt:kt + 1])

        # ---- Phase 2: distances ---------------------------------------
        with tc.tile_pool(name="p2ps", bufs=2, space="PSUM") as gps, \
             tc.tile_pool(name="p2sb", bufs=2) as d2p:
            for kt in range(KT):
                gp = gps.tile([128, QC], f32)
                nc.tensor.matmul(gp, lhsT=klocT[:, kt * 128:(kt + 1) * 128],
                                 rhs=qlocT[:, :], start=True, stop=True)
                d2 = d2p.tile([128, QC], f32)
                nc.vector.tensor_scalar(d2, gp, nk[:, kt:kt + 1],
                                        DCUT * DCUT, OP.add, OP.min)
                nc.scalar.activation(dc16[:, kt, :], d2, AF.Sqrt)

        nc.sync.dma_start(out=wo, in_=t_in["wo"][:, :, :])

        # ---- Phase 3: attention per head ------------------------------
        with tc.tile_pool(name="sps", bufs=5, space="PSUM") as sps, \
             tc.tile_pool(name="ctxps", bufs=2, space="PSUM") as cps, \
             tc.tile_pool(name="rbps", bufs=1, space="PSUM") as rps, \
             tc.tile_pool(name="p3x", bufs=3) as xp, \
             tc.tile_pool(name="p3e", bufs=3) as ep, \
             tc.tile_pool(name="p3p", bufs=4) as ptp, \
             tc.tile_pool(name="p3r", bufs=2) as rp:
            for h in range(H):
                ctxp = cps.tile([65, QC], f32, tag="ctx")
                for g in range(4):
                    dcg = dc16[:, 4 * g:4 * g + 4, :]
                    t1 = xp.tile([128, 4, QC], f16, tag="t1")
                    nc.vector.tensor_scalar_add(t1, dcg, coeffs[:, 4 * h:4 * h + 1])
                    x1 = xp.tile([128, 4, QC], f16, tag="x1")
                    nc.vector.tensor_tensor(out=x1, in0=t1, in1=dcg, op=OP.mult)
                    t2 = xp.tile([128, 4, QC], f16, tag="t2")
                    nc.vector.tensor_scalar_add(t2, x1, coeffs[:, 4 * h + 1:4 * h + 2])
                    x2 = xp.tile([128, 4, QC], f16, tag="x2")
                    nc.vector.tensor_tensor(out=x2, in0=t2, in1=dcg, op=OP.mult)
                    ee = ep.tile([128, 4, QC], f16, tag="ee")
                    nc.scalar.activation(ee, x2, AF.Exp,
                                         bias=coeffs[:, 4 * h + 3:4 * h + 4],
                                         scale=coeffs[:, 4 * h + 2:4 * h + 3])
                    for j in range(4):
                        kt = 4 * g + j
                        po = (h % 2) * 64
                        sp = sps.tile([128, QC], f32, tag="sp")
                        nc.tensor.matmul(sp, lhsT=kT[po:po + 64, h // 2,
                                                     kt * 128:(kt + 1) * 128],
                                         rhs=qT[po:po + 64, h // 2, :],
                                         start=True, stop=False)
                        nc.tensor.matmul(sp, lhsT=ident, rhs=ee[:, j, :],
                                         start=False, stop=True)
                        pt = ptp.tile([128, QC], f16, tag="pt")
                        nc.scalar.activation(pt, sp, AF.Exp,
                                             bias=kbias[:, kt:kt + 1])
                        nc.tensor.matmul(ctxp, lhsT=v_aug[:, kt, h, :], rhs=pt,
                                         start=(kt == 0), stop=(kt == KT - 1))
                scop = rp.tile([1, QC], f32, tag="scop")
                nc.vector.tensor_copy(scop, ctxp[64:65, :])
                rinv = rp.tile([1, QC], f32, tag="rinv")
                nc.vector.reciprocal_approx_fast(rinv, scop)
                if dbg:
                    dsum = rp.tile([1, QC], f32, tag="dsum")
                    nc.vector.tensor_copy(dsum, ctxp[64:65, :])
                    nc.sync.dma_start(out=t_dsums[h:h + 1, :], in_=dsum)
                rbp = rps.tile([64, QC], f32, tag="rb")
                nc.tensor.matmul(rbp, lhsT=onesf, rhs=rinv, start=True, stop=True)
                rb = rp.tile([64, QC], f32, tag="rbsb")
                nc.vector.tensor_copy(rb, rbp)
                nc.vector.tensor_tensor(out=ctx_sb[:, h, :], in0=ctxp[0:64, :],
                                        in1=rb, op=OP.mult)

        if dbg:
            nc.sync.dma_start(out=t_dqt[:, :, :], in_=qT)
            nc.sync.dma_start(out=t_ddc[:, :, :], in_=dc16)
            nc.sync.dma_start(out=t_dctx[:, :, :], in_=ctx_sb)

        # ---- Phase 4: output projection -------------------------------
        with tc.tile_pool(name="ops", bufs=2, space="PSUM") as ops, \
             tc.tile_pool(name="osb", bufs=2) as osb:
            for qc in range(4):
                op = ops.tile([128, DM], f32, tag="op")
                for h in range(H):
                    nc.tensor.matmul(op, lhsT=ctx_sb[:, h, qc * 128:(qc + 1) * 128],
                                     rhs=wo[:, h, :], start=(h == 0), stop=(h == H - 1))
                ot = osb.tile([128, DM], f32, tag="ot")
                nc.vector.tensor_copy(ot, op)
                nc.sync.dma_start(out=t_out[qc, :, :], in_=ot)

    nc.compile()
    return nc


_NC_CACHE = None


def _get_nc():
    global _NC_CACHE
    if _NC_CACHE is None:
        _NC_CACHE = _build_nc()
    return _NC_CACHE


def _prepare_in_maps(qs, ks, vs, qs_locs, ks_locs, Wq, bq, Wk, bk, Wv, bv,
                     Wo, bo, a, b, c, valid_lens):
    qs, ks, vs = np.asarray(qs, np.float32), np.asarray(ks, np.float32), np.asarray(vs, np.float32)
    qs_locs, ks_locs = np.asarray(qs_locs, np.float32), np.asarray(ks_locs, np.float32)
    vl = np.asarray(valid_lens).astype(np.int64)

    coefs = _fit_lnbias(np.asarray(a), np.asarray(b), np.asarray(c))
    crow = np.zeros(4 * H, np.float32)
    for h in range(H):
        s = coefs[h, DEG]
        crow[4 * h + 0] = coefs[h, 2] / s
        crow[4 * h + 1] = coefs[h, 1] / s
        crow[4 * h + 2] = s
        crow[4 * h + 3] = coefs[h, 0]
    coeffs_t = np.tile(crow, (128, 1)).astype(np.float32)

    def aug(x, last_row):
        A = np.zeros((NCB * 128, x.shape[1]), np.float32)
        A[:x.shape[0]] = x
        A[x.shape[0]] = last_row
        return _chunk(A.astype(np.float16))

    def chunk4(x):
        return np.ascontiguousarray(
            np.asarray(x, np.float32).astype(np.float16).reshape(4, 128, x.shape[1]).transpose(1, 0, 2))

    # bk shifts every score of a given query equally -> softmax-invariant, dropped.
    # bv is folded into bo on the host: (ctx + bv) @ Wo = ctx @ Wo + bv @ Wo.
    wq_c = aug(np.asarray(Wq, np.float32) / 8.0, np.asarray(bq, np.float32) / 8.0)
    wk_c = chunk4(np.asarray(Wk, np.float32))
    wv_c = chunk4(np.asarray(Wv, np.float32))
    wo_c = np.ascontiguousarray(
        np.asarray(Wo, np.float32).astype(np.float16).reshape(H, 64, DM).transpose(1, 0, 2))
    ident = np.eye(128, dtype=np.float16)
    onesf = np.ones((1, 64), np.float32)

    in_maps = []
    for i in range(8):
        bi, qo = i // 4, (i % 4) * QC
        qsl = qs[bi, qo:qo + QC]
        ksl, vsl = ks[bi], vs[bi]
        ql, kl = qs_locs[bi, qo:qo + QC], ks_locs[bi]
        qlocT = np.stack([ql[:, 0], ql[:, 1], (ql ** 2).sum(1),
                          np.zeros(QC, np.float32)]).astype(np.float32)
        klocT = np.stack([-2.0 * kl[:, 0], -2.0 * kl[:, 1],
                          np.ones(K, np.float32), np.zeros(K, np.float32)]).astype(np.float32)
        # +4e-4 keeps d^2 strictly positive under fp32 matmul cancellation
        # (sqrt(negative) = NaN); the d shift of <=0.02 near d=0 is harmless.
        nk_t = np.ascontiguousarray(
            ((kl ** 2).sum(1) + 4e-4).reshape(KT, 128).T.astype(np.float32))
        kb_t = np.full((128, KT), -SHIFT, np.float32)
        vm = (np.arange(K) < vl[bi]).astype(np.float32)
        vm_t = np.ascontiguousarray(vm.reshape(KT, 128).T)
        in_maps.append({
            "qsT": aug(qsl.T, 1.0), "ksT": chunk4(ksl.T), "vsT": chunk4(vsl.T),
            "wq": wq_c, "wk": wk_c, "wv": wv_c, "wo": wo_c,
            "qlocT": qlocT, "klocT": klocT, "nk": nk_t, "kbias": kb_t, "vmask": vm_t,
            "coeffs": coeffs_t, "ident": ident, "onesf": onesf,
        })
    return in_maps


def kernel(qs, ks, vs, qs_locs, ks_locs, Wq, bq, Wk, bk, Wv, bv, Wo, bo,
           a, b, c, valid_lens, **_unused):
    import os
    in_maps = _prepare_in_maps(qs, ks, vs, qs_locs, ks_locs, Wq, bq, Wk, bk,
                               Wv, bv, Wo, bo, a, b, c, valid_lens)
    nc = _get_nc()
    trace = bool(os.environ.get("KERNEL_TRACE"))
    res = run_bass_kernel_spmd(nc, in_maps, core_ids=list(range(8)), trace=trace)
    if trace and res.exec_time_ns is not None:
        print(f"HW exec time: {res.exec_time_ns} ns")

    bo32 = (np.asarray(bo, np.float32) +
            np.asarray(bv, np.float32) @ np.asarray(Wo, np.float32))
    out = np.zeros((B, Q, DM), np.float32)
    for i in range(8):
        bi, qo = i // 4, (i % 4) * QC
        out[bi, qo:qo + QC] = res.results[i]["out"].reshape(QC, DM) + bo32[None, :]
    return out
